# revision 5
# baseline (speedup 1.0000x reference)
"""Trainium2 Bass kernel for pair-biased gated attention (nn_AttentionCpp).

Reference computation (S=2048, C=768, H=16 heads, D=48):
    q = (x @ Wq + bq) * D**-0.5 ; k = x @ Wk ; v = x @ Wv
    logits[h,q,k] = q_h . k_h + pair_logits[h,q,k]   (masked over k)
    o = softmax_k(logits) @ v ;  out = sigmoid(x @ Wg) * o

Sharding: tensor-parallel over heads. Each of the 8 cores owns 2 heads:
column-slices of Wq/Wk/Wv/Wg and pair_logits[2i:2i+2]. No reduction is
needed; the host concatenates the per-core [S, 96] outputs.

Per-core device schedule (all matmul operands bf16, f32 accumulation):
 - q^T,k^T per head [48,S] from Wq/Wk tiles (lhsT) x x^T tiles (rhs)
 - v, gate in natural [S,96] orientation from x^T tiles (lhsT) x W (rhs)
 - per head, per 128-wide k-tile: scores^T[k,q] = k^T.T @ q^T in PSUM,
   then pair is accumulated INTO the same PSUM via transpose-matmuls
   (lhsT=pair natural chunk, rhs=identity), exp on ScalarE with the mask
   as a per-partition bias, PV accumulated as o^T[dv,q] with a ones
   column appended to v giving the softmax denominator for free
 - finalize: o^T -> SBUF, PE-transpose back to natural, reciprocal of
   the denominator column, scale, multiply by gate, DMA out.

exp skips max-subtraction: logits here are O(+-10) so fp32 exp is safe.
"""

import numpy as np

S, C, H, D = 2048, 768, 16, 48
N_CORES = 8
HPC = H // N_CORES  # heads per core = 2
G = HPC * D         # output columns per core = 96
KT = S // 128       # 16 k-tiles
QC = S // 128       # 16 q-chunks
NEG_INF = -1e9

_PATCHED = False
_NC_CACHE = None


def _patch_tile():
    """Split >1-wait sync_info across EventSemaphore instructions.

    This container's walrus rejects instructions carrying more than one
    sem-wait ("Too many sync wait commands"), but Tile's semaphore
    assignment can attach several. Hoisting the excess onto EventSemaphore
    instructions inserted immediately before (same engine) is equivalent:
    waits execute on the issuing sequencer in program order.
    """
    global _PATCHED
    if _PATCHED:
        return
    _PATCHED = True
    import concourse.mybir as mybir
    import concourse.tile as tile_mod

    CAP_DEFAULT, CAP_EVENTSEM = 1, 2

    def split_excess_waits(nc):
        for f in nc.m.functions:
            for blk in f.blocks:
                out, changed = [], False
                for inst in blk.instructions:
                    si = inst.sync_info
                    cap = (
                        CAP_EVENTSEM
                        if isinstance(inst, mybir.InstEventSemaphore)
                        else CAP_DEFAULT
                    )
                    if si is not None and si.on_wait and len(si.on_wait) > cap:
                        extra = list(si.on_wait[cap:])
                        del si.on_wait[cap:]
                        for i in range(0, len(extra), CAP_EVENTSEM):
                            ev = mybir.InstEventSemaphore(
                                name=nc.get_next_instruction_name(),
                                engine=inst.engine,
                                ins=[],
                                outs=[],
                                sync_info=mybir.SyncInfo(
                                    on_wait=extra[i : i + CAP_EVENTSEM], on_update=[]
                                ),
                            )
                            nc.register_instruction(ev, overwrite=True)
                            out.append(ev)
                        changed = True
                    out.append(inst)
                if changed:
                    blk.instructions = out

    orig_exit = tile_mod.TileContext.__exit__

    def _exit(self, *args):
        r = orig_exit(self, *args)
        split_excess_waits(self.nc)
        return r

    tile_mod.TileContext.__exit__ = _exit
    tile_mod.TileContext._ant_wait_split = True


def _build_nc():
    import concourse.bass as bass
    import concourse.mybir as mybir
    from concourse.tile import TileContext

    bf = mybir.dt.bfloat16
    f32 = mybir.dt.float32
    AF = mybir.ActivationFunctionType

    nc = bass.Bass()
    d_xt = nc.dram_tensor("xt", [C, S], bf, kind="ExternalInput")
    d_wq = nc.dram_tensor("wq", [C, G], bf, kind="ExternalInput")
    d_wk = nc.dram_tensor("wk", [C, G], bf, kind="ExternalInput")
    d_wv = nc.dram_tensor("wv", [C, G], bf, kind="ExternalInput")
    d_wg = nc.dram_tensor("wg", [C, G], bf, kind="ExternalInput")
    d_bq = nc.dram_tensor("bqv", [D, HPC], f32, kind="ExternalInput")
    d_pair = nc.dram_tensor("pair", [HPC, S, S], bf, kind="ExternalInput")
    d_ident = nc.dram_tensor("ident", [128, 128], bf, kind="ExternalInput")
    d_identf = nc.dram_tensor("identf", [128, 128], f32, kind="ExternalInput")
    d_mb = nc.dram_tensor("mb", [128, KT], f32, kind="ExternalInput")
    d_out = nc.dram_tensor("out", [S, G], f32, kind="ExternalOutput")

    CT = C // 128  # 6 contraction tiles

    with TileContext(nc) as tc:
        with tc.tile_pool(name="const", bufs=1) as const, \
             tc.tile_pool(name="pairp", bufs=3) as pairp, \
             tc.tile_pool(name="probsp", bufs=3) as probsp, \
             tc.tile_pool(name="sbsmall", bufs=4) as sbsmall, \
             tc.tile_pool(name="osb", bufs=2) as osbp:

            t_ident = const.tile([128, 128], bf)
            t_identf = const.tile([128, 128], f32)
            t_mb = const.tile([128, KT], f32)
            t_bq = const.tile([D, HPC], f32)
            nc.sync.dma_start(t_ident[:], d_ident[:])
            nc.sync.dma_start(t_identf[:], d_identf[:])
            nc.sync.dma_start(t_mb[:], d_mb[:])
            nc.sync.dma_start(t_bq[:], d_bq[:])

            # persistent activations
            t_qT = [const.tile([D, S], bf, tag=f"qT{h}", name=f"qT{h}") for h in range(HPC)]
            t_kT = [const.tile([D, S], bf, tag=f"kT{h}", name=f"kT{h}") for h in range(HPC)]
            t_vn = const.tile([128, KT, HPC, D + 1], bf)  # v natural + ones col
            t_gate = const.tile([128, QC, G], bf)
            t_out = const.tile([128, QC, G], f32)

            # ---- phase 1: projections ----
            with tc.tile_pool(name="xw", bufs=1) as xw, \
                 tc.tile_pool(name="ps_p", bufs=1, space="PSUM") as ps_p, \
                 tc.tile_pool(name="ps_s", bufs=2, space="PSUM") as ps_s:
                t_x = xw.tile([128, CT, S], bf)
                nc.sync.dma_start(t_x[:], d_xt.rearrange("(ct p) s -> p ct s", p=128))
                t_w = {}
                for name, dram in (("wq", d_wq), ("wk", d_wk), ("wv", d_wv), ("wg", d_wg)):
                    t_w[name] = xw.tile([128, CT, G], bf, tag=name, name=f"w_{name}")
                    nc.sync.dma_start(
                        t_w[name][:], dram.rearrange("(ct p) g -> p ct g", p=128)
                    )

                # q^T / k^T per head: [48, S] = W_slice.T @ x^T
                for h in range(HPC):
                    for name, dst in (("wq", t_qT[h]), ("wk", t_kT[h])):
                        pp = ps_p.tile([D, S], f32, tag="proj")
                        for ct in range(CT):
                            for qc in range(4):
                                nc.tensor.matmul(
                                    pp[:, qc * 512:(qc + 1) * 512],
                                    t_w[name][:, ct, h * D:(h + 1) * D],
                                    t_x[:, ct, qc * 512:(qc + 1) * 512],
                                    start=(ct == 0), stop=(ct == CT - 1),
                                )
                        if name == "wq":
                            nc.scalar.add(dst[:], pp[:], t_bq[:, h:h + 1])
                        else:
                            nc.vector.tensor_copy(dst[:], pp[:])

                # v natural per k-tile; gate natural per q-chunk
                for i in range(KT):
                    pv = ps_s.tile([128, G], f32, tag="pv")
                    pg = ps_s.tile([128, G], f32, tag="pg")
                    for ct in range(CT):
                        nc.tensor.matmul(
                            pv[:], t_x[:, ct, i * 128:(i + 1) * 128],
                            t_w["wv"][:, ct, :],
                            start=(ct == 0), stop=(ct == CT - 1),
                        )
                        nc.tensor.matmul(
                            pg[:], t_x[:, ct, i * 128:(i + 1) * 128],
                            t_w["wg"][:, ct, :],
                            start=(ct == 0), stop=(ct == CT - 1),
                        )
                    for h in range(HPC):
                        nc.vector.tensor_copy(
                            t_vn[:, i, h, 0:D], pv[:, h * D:(h + 1) * D]
                        )
                        nc.vector.memset(t_vn[:, i, h, D:D + 1], 1.0)
                    nc.scalar.activation(
                        t_gate[:, i, :], pg[:], AF.Sigmoid
                    )

            # ---- phase 2+3: attention per head ----
            with tc.tile_pool(name="ps_sc", bufs=2, space="PSUM") as ps_sc, \
                 tc.tile_pool(name="ps_o", bufs=1, space="PSUM") as ps_o:
                for h in range(HPC):
                    t_po = ps_o.tile([D + 1, S], f32, tag="po")
                    for kc in range(KT // 2):  # 256-col pair chunks (2 k-tiles)
                        t_pair = pairp.tile([128, QC, 256], bf, tag="pair")
                        nc.sync.dma_start(
                            t_pair[:],
                            d_pair[h].rearrange("(qc p) k -> p qc k", p=128)
                            [:, :, kc * 256:(kc + 1) * 256],
                        )
                        for kl in range(2):
                            kt = kc * 2 + kl
                            for half in range(2):
                                s = ps_sc.tile([128, 1024], f32, tag="sc", name="s")
                                for qc in range(2):
                                    nc.tensor.matmul(
                                        s[:, qc * 512:(qc + 1) * 512],
                                        t_kT[h][:, kt * 128:(kt + 1) * 128],
                                        t_qT[h][:, (half * 2 + qc) * 512:(half * 2 + qc + 1) * 512],
                                        start=True, stop=False,
                                    )
                                for qb in range(8):
                                    qcg = half * 8 + qb
                                    nc.tensor.matmul(
                                        s[:, qb * 128:(qb + 1) * 128],
                                        t_pair[:, qcg, kl * 128:(kl + 1) * 128],
                                        t_ident[:],
                                        start=False, stop=(qb % 4 == 3),
                                    )
                                probs = probsp.tile([128, 1024], bf, tag="probs", name="probs")
                                nc.scalar.activation(
                                    probs[:], s[:], AF.Exp,
                                    bias=t_mb[:, kt:kt + 1], scale=1.0,
                                )
                                for qc in range(2):
                                    nc.tensor.matmul(
                                        t_po[:, (half * 2 + qc) * 512:(half * 2 + qc + 1) * 512],
                                        t_vn[:, kt, h, :],
                                        probs[:, qc * 512:(qc + 1) * 512],
                                        start=(kt == 0), stop=(kt == KT - 1),
                                    )
                    # finalize head h
                    o_sb = osbp.tile([D + 1, S], f32, tag="o_sb", name="o_sb")
                    nc.scalar.copy(o_sb[:], t_po[:])
                    for qc in range(QC):
                        ot = ps_sc.tile([128, D + 1], f32, tag="sc", name="ot")
                        nc.tensor.transpose(
                            ot[:], o_sb[:, qc * 128:(qc + 1) * 128],
                            t_identf[0:D + 1, 0:D + 1],
                        )
                        recip = sbsmall.tile([128, 1], f32, tag="recip", name="recip")
                        nc.vector.reciprocal(recip[:], ot[:, D:D + 1])
                        o_n = sbsmall.tile([128, D], bf, tag="o_n", name="o_n")
                        nc.vector.tensor_scalar_mul(o_n[:], ot[:, 0:D], recip[:])
                        nc.vector.tensor_mul(
                            t_out[:, qc, h * D:(h + 1) * D],
                            o_n[:], t_gate[:, qc, h * D:(h + 1) * D],
                        )

            nc.sync.dma_start(
                d_out.rearrange("(qc p) g -> p qc g", p=128), t_out[:]
            )
    return nc


def kernel(x, mask, pair_logits, Wq, bq, Wk, Wv, Wg):
    import ml_dtypes

    _patch_tile()
    global _NC_CACHE
    if _NC_CACHE is None:
        _NC_CACHE = _build_nc()
    nc = _NC_CACHE
    from concourse.bass_utils import run_bass_kernel_spmd

    bf = ml_dtypes.bfloat16
    scale = np.float32(D ** -0.5)
    xt = np.ascontiguousarray(x.astype(np.float32).T).astype(bf)
    wq_s = (Wq.astype(np.float32) * scale).astype(bf)
    wk_s = Wk.astype(bf)
    wv_s = Wv.astype(bf)
    wg_s = Wg.astype(bf)
    bq_s = (bq.astype(np.float32) * scale)
    maskbias = np.where(mask, 0.0, NEG_INF).astype(np.float32)
    mb_t = np.ascontiguousarray(maskbias.reshape(KT, 128).T)
    ident = np.eye(128, dtype=bf)
    identf = np.eye(128, dtype=np.float32)

    in_maps = []
    for i in range(N_CORES):
        cols = slice(i * G, (i + 1) * G)
        in_maps.append({
            "xt": xt,
            "wq": np.ascontiguousarray(wq_s[:, cols]),
            "wk": np.ascontiguousarray(wk_s[:, cols]),
            "wv": np.ascontiguousarray(wv_s[:, cols]),
            "wg": np.ascontiguousarray(wg_s[:, cols]),
            "bqv": np.ascontiguousarray(bq_s[cols].reshape(HPC, D).T),
            "pair": np.ascontiguousarray(pair_logits[i * HPC:(i + 1) * HPC]).astype(bf),
            "ident": ident,
            "identf": identf,
            "mb": mb_t,
        })

    res = run_bass_kernel_spmd(nc, in_maps, core_ids=list(range(N_CORES)))
    out = np.empty((S, C), np.float32)
    for i in range(N_CORES):
        out[:, i * G:(i + 1) * G] = res.results[i]["out"]
    return out
